# revision 1
# baseline (speedup 1.0000x reference)
"""GCN sampling (NodeFlow) kernel for 8 TRN2 NeuronCores.

Geometry (hardcoded per problem spec):
  N0=409600 nodes x 512 feats, layer0: 40960 dst x fanout 10, W1 [512,256]+relu,
  layer1: 4096 dst x fanout 10, W2 [256,64].

Strategy: shard layer-1 dst nodes across 8 cores (512 each). Each core pulls,
for each of its 5120 layer-1 edges, the 10 layer-0 feature rows of that edge's
src h-row (indices precomputed on host; h-rows deliberately duplicated per
layer-1 edge so BOTH segment-means become fixed-stride pooling, no on-device
gather for layer 1 and no cross-core communication). Per core:
  indirect-DMA gather 51200 rows (100 MiB) -> pool(10) -> [5120,512]
  -> PE transpose -> W1 matmul + relu -> [256hid x 5120] (hid on partitions)
  -> pool(10) along free dim -> [256 x 512] -> W2 matmul -> [512, 64].
1/10 mean factors are folded into W1, W2 on the host.
"""

import sys

sys.path.insert(0, "/opt/trn_rl_repo")

from contextlib import ExitStack

import numpy as np

N0, N1, N2 = 409600, 40960, 4096
F = 10                      # fanout
IN_F, HID, NCLS = 512, 256, 64
NC_N = 8                    # cores
DST_PC = N2 // NC_N         # 512 dst nodes per core
GRP_PC = DST_PC * F         # 5120 h-rows (groups) per core
BLK = 128                   # groups per gather block (partition dim)
NBLK = GRP_PC // BLK        # 40 blocks
SB = 4                      # blocks per matmul superblock (512 rows)
NSB = NBLK // SB            # 10 superblocks

_BUILT = None


def _legalize_waits(bir: bytes) -> bytes:
    """This container's walrus supports exactly ONE sync-wait per instruction.
    Split every multi-wait instruction: keep the last wait, hoist the others
    onto single-wait EventSemaphore instructions inserted just before it on
    the same engine (same semantics: engine sequencer blocks in order)."""
    import orjson

    j = orjson.loads(bir)
    n_new = 0
    for fn in j["functions"]:
        for bb in fn["blocks"]:
            insts = bb["instructions"]
            out = []
            for inst in insts:
                si = inst.get("sync_info")
                waits = si.get("on_wait") if si else None
                if waits and len(waits) > 1:
                    for w in waits[:-1]:
                        n_new += 1
                        out.append({
                            "debug": inst.get("debug", 0),
                            "engine": inst["engine"],
                            "ins": [],
                            "name": f"{inst['name']}_esw{n_new}",
                            "opcode": "EventSemaphore",
                            "outs": [],
                            "sync_info": {"on_update": [], "on_wait": [w]},
                        })
                    si["on_wait"] = [waits[-1]]
                out.append(inst)
            bb["instructions"] = out
    return orjson.dumps(j)


def _install_patch():
    import concourse.bass as bass

    if getattr(bass.Bass, "_gcn_wait_patch", False):
        return
    orig = bass.Bass.to_json_bytes

    def to_json_bytes(self, *a, **kw):
        return _legalize_waits(orig(self, *a, **kw))

    bass.Bass.to_json_bytes = to_json_bytes
    bass.Bass._gcn_wait_patch = True


def build_nc():
    """Build the SPMD Bass program (identical on all cores)."""
    _install_patch()
    import concourse.bass as bass
    import concourse.tile as tile
    from concourse import mybir
    from concourse.masks import make_identity

    f32 = mybir.dt.float32
    nc = bass.Bass("TRN2", target_bir_lowering=False, debug=False,
                   num_devices=NC_N, num_swdge_queues=4)

    feat = nc.dram_tensor("feat", [N0, IN_F], f32, kind="ExternalInput")
    w1 = nc.dram_tensor("w1", [IN_F, HID], f32, kind="ExternalInput")
    b1 = nc.dram_tensor("b1", [HID], f32, kind="ExternalInput")
    w2 = nc.dram_tensor("w2", [HID, NCLS], f32, kind="ExternalInput")
    b2 = nc.dram_tensor("b2", [NCLS], f32, kind="ExternalInput")
    idx = nc.dram_tensor("idx", [BLK, NBLK * F], mybir.dt.int32,
                         kind="ExternalInput")
    out = nc.dram_tensor("out", [DST_PC, NCLS], f32, kind="ExternalOutput")

    with tile.TileContext(nc) as tc, ExitStack() as ctx:
        consts = ctx.enter_context(tc.tile_pool(name="consts", bufs=1))
        gpool = ctx.enter_context(tc.tile_pool(name="gather", bufs=3))
        tpool = ctx.enter_context(tc.tile_pool(name="temps", bufs=2))
        hpool = ctx.enter_context(tc.tile_pool(name="hhT", bufs=2))
        h1pool = ctx.enter_context(tc.tile_pool(name="hh1T", bufs=1))
        opool = ctx.enter_context(tc.tile_pool(name="outs", bufs=2))
        ps_tr = ctx.enter_context(tc.tile_pool(name="ps_tr", bufs=3, space="PSUM"))
        ps_mm = ctx.enter_context(tc.tile_pool(name="ps_mm", bufs=2, space="PSUM"))
        ps_o = ctx.enter_context(tc.tile_pool(name="ps_o", bufs=2, space="PSUM"))

        idx_t = consts.tile([BLK, NBLK * F], mybir.dt.int32)
        nc.gpsimd.dma_start(idx_t[:], idx.ap())
        w1t = consts.tile([128, 4 * HID], f32)       # col fc*256+h = w1[fc*128+p, h]
        nc.gpsimd.dma_start(w1t[:].rearrange("p (f h) -> p f h", f=4),
                            w1.ap().rearrange("(f p) h -> p f h", f=4))
        w2t = consts.tile([128, 2 * NCLS], f32)      # col c*64+n = w2[c*128+p, n]
        nc.gpsimd.dma_start(w2t[:].rearrange("p (c n) -> p c n", c=2),
                            w2.ap().rearrange("(c p) n -> p c n", c=2))
        b1t = consts.tile([128, 2], f32)             # col h = b1[h*128+p]
        nc.gpsimd.dma_start(b1t[:], b1.ap().rearrange("(h p) -> p h", h=2))
        b2t = consts.tile([1, NCLS], f32)
        nc.gpsimd.dma_start(b2t[:], b2.ap().unsqueeze(0))
        ident = consts.tile([128, 128], f32)
        make_identity(nc, ident[:])

        # hh1T[hc]: [128 hid, 5120 rows], rows = layer-1 edges
        hh1T = [h1pool.tile([128, GRP_PC], f32, tag=f"hh1T{hc}",
                               name=f"hh1T{hc}") for hc in range(2)]

        hhT_sb = None
        for b in range(NBLK):
            if b % SB == 0:
                hhT_sb = [hpool.tile([128, SB * 128], f32, tag=f"hhT{fc}",
                                     name=f"hhT{fc}_{b}") for fc in range(4)]
            # gather 128 groups x 512 feats per edge-slot k (HW indirect DMA
            # honors exactly one index per partition per instruction)
            g = []
            for k in range(F):
                gk = gpool.tile([BLK, IN_F], mybir.dt.float32, tag=f"g{k}",
                                name=f"g{k}_{b}")
                gi = nc.gpsimd.indirect_dma_start(
                    out=gk[:], out_offset=None, in_=feat.ap(),
                    in_offset=bass.IndirectOffsetOnAxis(
                        ap=idx_t[:, b * F + k:b * F + k + 1], axis=0),
                )
                q = (b * F + k) % 4
                if q:  # spread gathers over the 4 SWDGE queues
                    gi.ins.queue = f"qPoolDynamic{q}"
                g.append(gk)
            # pool the 10 edges (tree adds on DVE)
            s0 = tpool.tile([BLK, IN_F], f32, tag="s0")
            s1 = tpool.tile([BLK, IN_F], f32, tag="s1")
            s2 = tpool.tile([BLK, IN_F], f32, tag="s2")
            s3 = tpool.tile([BLK, IN_F], f32, tag="s3")
            s4 = tpool.tile([BLK, IN_F], f32, tag="s4")
            hs = tpool.tile([BLK, IN_F], f32, tag="hs")
            E = lambda k: g[k][:]
            nc.vector.tensor_add(s0[:], E(0), E(1))
            nc.vector.tensor_add(s1[:], E(2), E(3))
            nc.vector.tensor_add(s2[:], E(4), E(5))
            nc.vector.tensor_add(s3[:], E(6), E(7))
            nc.vector.tensor_add(s4[:], E(8), E(9))
            nc.vector.tensor_add(s0[:], s0[:], s1[:])
            nc.vector.tensor_add(s2[:], s2[:], s3[:])
            nc.vector.tensor_add(s0[:], s0[:], s2[:])
            nc.vector.tensor_add(hs[:], s0[:], s4[:])
            # transpose to [feat, rows] chunks
            col = (b % SB) * 128
            for fc in range(4):
                ptr = ps_tr.tile([128, 128], f32, tag="ptr", space="PSUM")
                nc.tensor.transpose(ptr[:], hs[:, fc * 128:(fc + 1) * 128],
                                    ident[:])
                nc.vector.tensor_copy(hhT_sb[fc][:, col:col + 128], ptr[:])
            if b % SB == SB - 1:
                sb = b // SB
                rows = slice(sb * SB * 128, (sb + 1) * SB * 128)
                for hc in range(2):
                    pm = ps_mm.tile([128, SB * 128], f32, tag="pm", space="PSUM")
                    for fc in range(4):
                        nc.tensor.matmul(
                            pm[:],
                            lhsT=w1t[:, fc * HID + hc * 128: fc * HID + hc * 128 + 128],
                            rhs=hhT_sb[fc][:],
                            start=(fc == 0), stop=(fc == 3),
                        )
                    nc.scalar.activation(hh1T[hc][:, rows], pm[:],
                                         mybir.ActivationFunctionType.Relu,
                                         bias=b1t[:, hc:hc + 1])

        # layer-1 pooling along free dim: [128, 5120] -> [128, 512]
        g2 = []
        for hc in range(2):
            v = hh1T[hc][:].rearrange("p (r k) -> p r k", k=F)
            t0 = tpool.tile([128, DST_PC], f32, tag="p2a", bufs=1)
            t1 = tpool.tile([128, DST_PC], f32, tag="p2b", bufs=1)
            t2 = tpool.tile([128, DST_PC], f32, tag="p2c", bufs=1)
            t3 = tpool.tile([128, DST_PC], f32, tag="p2d", bufs=1)
            t4 = tpool.tile([128, DST_PC], f32, tag="p2e", bufs=1)
            gg = tpool.tile([128, DST_PC], f32, tag=f"g2_{hc}", bufs=1)
            V = lambda k: v[:, :, k]
            nc.vector.tensor_add(t0[:], V(0), V(1))
            nc.vector.tensor_add(t1[:], V(2), V(3))
            nc.vector.tensor_add(t2[:], V(4), V(5))
            nc.vector.tensor_add(t3[:], V(6), V(7))
            nc.vector.tensor_add(t4[:], V(8), V(9))
            nc.vector.tensor_add(t0[:], t0[:], t1[:])
            nc.vector.tensor_add(t2[:], t2[:], t3[:])
            nc.vector.tensor_add(t0[:], t0[:], t2[:])
            nc.vector.tensor_add(gg[:], t0[:], t4[:])
            g2.append(gg)

        # final matmul: out[r, n] = sum_h g2[h, r] * w2[h, n] + b2
        for rb in range(DST_PC // 128):
            po = ps_o.tile([128, NCLS], f32, tag="po", space="PSUM")
            for hc in range(2):
                nc.tensor.matmul(
                    po[:],
                    lhsT=g2[hc][:, rb * 128:(rb + 1) * 128],
                    rhs=w2t[:, hc * NCLS:(hc + 1) * NCLS],
                    start=(hc == 0), stop=(hc == 1),
                )
            ot = opool.tile([128, NCLS], f32, tag="ot")
            nc.vector.tensor_copy(ot[:], po[:])
            nc.gpsimd.dma_start(out.ap()[rb * 128:(rb + 1) * 128, :], ot[:])

    return nc


def _get_nc():
    global _BUILT
    if _BUILT is None:
        _BUILT = build_nc()
    return _BUILT


def _prep_core_indices(src0, src1, core):
    s1 = src1[core * GRP_PC:(core + 1) * GRP_PC].astype(np.int64)
    G = src0[(s1[:, None] * F + np.arange(F)[None, :])]        # [5120, 10]
    return np.ascontiguousarray(
        G.reshape(NBLK, BLK, F).transpose(1, 0, 2).reshape(BLK, NBLK * F)
    ).astype(np.int32)


def _run(inputs, trace=False, trace_kwargs=None):
    from concourse.bass_utils import run_bass_kernel_spmd

    features = np.ascontiguousarray(inputs["features"], dtype=np.float32)
    w1s = np.ascontiguousarray(inputs["W1"], dtype=np.float32) / np.float32(F)
    w2s = np.ascontiguousarray(inputs["W2"], dtype=np.float32) / np.float32(F)
    b1 = np.ascontiguousarray(inputs["b1"], dtype=np.float32)
    b2 = np.ascontiguousarray(inputs["b2"], dtype=np.float32)
    src0 = np.asarray(inputs["src0"]).astype(np.int64)
    src1 = np.asarray(inputs["src1"]).astype(np.int64)

    in_maps = []
    for c in range(NC_N):
        in_maps.append({
            "feat": features, "w1": w1s, "b1": b1, "w2": w2s, "b2": b2,
            "idx": _prep_core_indices(src0, src1, c),
        })
    nc = _get_nc()
    kw = {}
    if trace:
        kw = {"trace": True, "trace_kwargs": trace_kwargs or {}}
    res = run_bass_kernel_spmd(nc, in_maps, list(range(NC_N)), **kw)
    full = np.concatenate([res.results[c]["out"] for c in range(NC_N)], axis=0)
    full = full + b2[None, :]
    return full, res


def kernel(features, W1, b1, W2, b2, src0, dst0, src1, dst1):
    ins = dict(features=features, W1=W1, b1=b1, W2=W2, b2=b2,
               src0=src0, dst0=dst0, src1=src1, dst1=dst1)
    d0 = np.asarray(dst0); d1 = np.asarray(dst1)
    fixed = (d0 == np.arange(N1 * F) // F).all() and \
            (d1 == np.arange(N2 * F) // F).all()
    if not fixed:
        # general (unexpected) dst pattern: numpy fallback for correctness
        f = np.asarray(features, dtype=np.float64)
        m = f[np.asarray(src0)]
        s = np.zeros((N1, IN_F)); np.add.at(s, d0, m)
        deg = np.bincount(d0, minlength=N1).clip(1)
        h = np.maximum(s / deg[:, None] @ np.asarray(W1) + np.asarray(b1), 0)
        m = h[np.asarray(src1)]
        s = np.zeros((N2, HID)); np.add.at(s, d1, m)
        deg = np.bincount(d1, minlength=N2).clip(1)
        return ((s / deg[:, None]) @ np.asarray(W2) + np.asarray(b2)
                ).astype(np.float32)
    out, _ = _run(ins)
    return out



# revision 12
# speedup vs baseline: 1.0582x; 1.0582x over previous
"""GCN sampling (NodeFlow) kernel for 8 TRN2 NeuronCores — v3.

Geometry (hardcoded per problem spec):
  N0=409600 nodes x 512 feats, layer0: 40960 dst x fanout 10, W1 [512,256]+relu,
  layer1: 4096 dst x fanout 10, W2 [256,64].

Strategy: shard layer-1 dst nodes across 8 cores (512 each). Each core pulls,
for each of its 5120 layer-1 edges, the 10 layer-0 feature rows of that edge's
src h-row (indices precomputed on host; h-rows deliberately duplicated per
layer-1 edge so BOTH segment-means become fixed-stride pooling, no on-device
gather for layer 1 and no cross-core communication). 1/10 mean factors are
folded into W1, W2 on the host.

The modeled bottleneck is the Pool engine's SWDGE descriptor generation for
the 400 indirect gathers (the HW DGE honors exactly one index per partition
per instruction — verified by probe; multi-index gathers return garbage for
slots >0, and the InstDMAGatherAnt custom path crashes this runtime). Per
gather: ~1040ns descriptor-gen vs 728ns DMA transfer, so ~415us Pool-bound.
Everything else is arranged to hide under it and to keep the head/tail of the
pipeline short:
  * block-0 indices are loaded as a separate tiny DMA so the first gather
    starts as early as possible; all constants load via HWDGE (SP engine),
    keeping Pool exclusively on gather descriptor generation.
  * per block of 128 edges: DVE pairwise-tree pools the 10 slots (f32),
    4 single-shot PE transposes produce h0^T, DVE downcasts PSUM->bf16,
    W1 matmul in bf16 (1 cycle/row) + ReLU per block.
  * hop-2 pooling is a per-block stride-2 pair-add (pairs never straddle
    blocks) plus a 5-block chunk tail (64 dst): 5->1 tree, W2 (bf16), store.
    The post-last-gather critical chain is one block's compute + one small
    chunk tail instead of a full-width epilogue.
"""

import sys

sys.path.insert(0, "/opt/trn_rl_repo")

from contextlib import ExitStack

import numpy as np

N0, N1, N2 = 409600, 40960, 4096
F = 10                      # fanout
IN_F, HID, NCLS = 512, 256, 64
NC_N = 8                    # cores
DST_PC = N2 // NC_N         # 512 dst nodes per core
GRP_PC = DST_PC * F         # 5120 h-rows (edges) per core
BLK = 128                   # edges per block (partition dim)
NBLK = GRP_PC // BLK        # 40 blocks
CHUNK_BLKS = 5              # blocks per output chunk (640 edges = 64 dst)
NCHUNK = NBLK // CHUNK_BLKS  # 8 output chunks
CHUNK_DST = BLK * CHUNK_BLKS // F   # 64 dst rows per chunk

_BUILT = None


def _legalize_waits(bir: bytes) -> bytes:
    """This container's walrus supports exactly ONE sync-wait per instruction.
    Split every multi-wait instruction: keep the last wait, hoist the others
    onto single-wait EventSemaphore instructions inserted just before it on
    the same engine (same semantics: engine sequencer blocks in order)."""
    import orjson

    j = orjson.loads(bir)
    n_new = 0
    for fn in j["functions"]:
        for bb in fn["blocks"]:
            insts = bb["instructions"]
            out = []
            for inst in insts:
                si = inst.get("sync_info")
                waits = si.get("on_wait") if si else None
                if waits and len(waits) > 1:
                    for w in waits[:-1]:
                        n_new += 1
                        out.append({
                            "debug": inst.get("debug", 0),
                            "engine": inst["engine"],
                            "ins": [],
                            "name": f"{inst['name']}_esw{n_new}",
                            "opcode": "EventSemaphore",
                            "outs": [],
                            "sync_info": {"on_update": [], "on_wait": [w]},
                        })
                    si["on_wait"] = [waits[-1]]
                out.append(inst)
            bb["instructions"] = out
    return orjson.dumps(j)


def _install_patch():
    import concourse.bass as bass

    if getattr(bass.Bass, "_gcn_wait_patch", False):
        return
    orig = bass.Bass.to_json_bytes

    def to_json_bytes(self, *a, **kw):
        return _legalize_waits(orig(self, *a, **kw))

    bass.Bass.to_json_bytes = to_json_bytes
    bass.Bass._gcn_wait_patch = True


def build_nc():
    """Build the SPMD Bass program (identical on all cores)."""
    _install_patch()
    import concourse.bass as bass
    import concourse.tile as tile
    from concourse import mybir

    f32 = mybir.dt.float32
    bf16 = mybir.dt.bfloat16
    nc = bass.Bass("TRN2", target_bir_lowering=False, debug=False,
                   num_devices=NC_N, num_swdge_queues=4)

    feat = nc.dram_tensor("feat", [N0, IN_F], f32, kind="ExternalInput")
    # host-prearranged weights: w1t[p, fc*256+h] = W1[fc*128+p, h]/10 (bf16)
    w1t_d = nc.dram_tensor("w1t", [128, 4 * HID], bf16, kind="ExternalInput")
    # b1t[p, hc] = b1[hc*128+p] (f32)
    b1t_d = nc.dram_tensor("b1t", [128, 2], f32, kind="ExternalInput")
    # w2t[p, c*64+n] = W2[c*128+p, n]/10 (bf16)
    w2t_d = nc.dram_tensor("w2t", [128, 2 * NCLS], bf16, kind="ExternalInput")
    identb_d = nc.dram_tensor("identb", [128, 128], f32, kind="ExternalInput")
    idx0_d = nc.dram_tensor("idx0", [BLK, F], mybir.dt.int32,
                            kind="ExternalInput")
    idxr_d = nc.dram_tensor("idxr", [BLK, (NBLK - 1) * F], mybir.dt.int32,
                            kind="ExternalInput")
    out = nc.dram_tensor("out", [DST_PC, NCLS], f32, kind="ExternalOutput")

    with tile.TileContext(nc) as tc, ExitStack() as ctx:
        consts = ctx.enter_context(tc.tile_pool(name="consts", bufs=1))
        gpool = ctx.enter_context(tc.tile_pool(name="gather", bufs=3))
        spool = ctx.enter_context(tc.tile_pool(name="pooled", bufs=2))
        hpool = ctx.enter_context(tc.tile_pool(name="hhT", bufs=2))
        h1pool = ctx.enter_context(tc.tile_pool(name="h1b", bufs=2))
        prpool = ctx.enter_context(tc.tile_pool(name="pairs", bufs=1))
        t2pool = ctx.enter_context(tc.tile_pool(name="tail", bufs=2))
        opool = ctx.enter_context(tc.tile_pool(name="outs", bufs=2))
        ps_tr = ctx.enter_context(tc.tile_pool(name="ps_tr", bufs=3, space="PSUM"))
        ps_mm = ctx.enter_context(tc.tile_pool(name="ps_mm", bufs=2, space="PSUM"))
        ps_o = ctx.enter_context(tc.tile_pool(name="ps_o", bufs=2, space="PSUM"))

        # block-0 indices first (tiny), so gather 0 starts ~earliest
        idx0_t = consts.tile([BLK, F], mybir.dt.int32)
        nc.sync.dma_start(idx0_t[:], idx0_d.ap())
        idxr_t = consts.tile([BLK, (NBLK - 1) * F], mybir.dt.int32)
        nc.sync.dma_start(idxr_t[:], idxr_d.ap())
        w1t = consts.tile([128, 4 * HID], bf16)
        nc.sync.dma_start(w1t[:], w1t_d.ap())
        b1t = consts.tile([128, 2], f32)
        nc.sync.dma_start(b1t[:], b1t_d.ap())
        w2t = consts.tile([128, 2 * NCLS], bf16)
        nc.sync.dma_start(w2t[:], w2t_d.ap())
        identb = consts.tile([128, 128], f32)
        nc.sync.dma_start(identb[:], identb_d.ap())

        # pair-sums of hop-2 slots: pairT[hc][p, b*64+i] = h1[e=2i]+h1[e=2i+1]
        pairT = [prpool.tile([128, NBLK * BLK // 2], bf16, tag=f"pairT{hc}",
                             name=f"pairT{hc}") for hc in range(2)]

        def emit_gathers(g, b, k0, k1):
            """Single-index gathers for block b, slots [k0, k1)."""
            for k in range(k0, k1):
                if b == 0:
                    iap = idx0_t[:, k:k + 1]
                else:
                    iap = idxr_t[:, (b - 1) * F + k:(b - 1) * F + k + 1]
                nc.gpsimd.indirect_dma_start(
                    out=g[:, k * IN_F:(k + 1) * IN_F], out_offset=None,
                    in_=feat.ap(),
                    in_offset=bass.IndirectOffsetOnAxis(ap=iap, axis=0),
                )

        def emit_transposes(b, hs):
            """4 single-shot PE transposes -> hhT (bf16 downcast on copy)."""
            hhT = [hpool.tile([128, BLK], bf16, tag=f"hhT{fc}",
                              name=f"hhT{fc}_{b}") for fc in range(4)]
            for fc in range(4):
                ptr = ps_tr.tile([128, 128], f32, tag="ptr", space="PSUM")
                nc.tensor.transpose(ptr[:], hs[:, fc * 128:(fc + 1) * 128],
                                    identb[:])
                nc.vector.tensor_copy(hhT[fc][:], ptr[:])
            return hhT

        def emit_gather_and_pt(b):
            """10 single-index gathers for block b + pool + transpose."""
            g = gpool.tile([BLK, F * IN_F], f32, tag="g", name=f"g_{b}")
            emit_gathers(g, b, 0, F)
            # DVE pairwise tree over the 10 slots (f32, wide strided ops)
            v = g[:].rearrange("p (r two f) -> p r two f", two=2, f=IN_F)
            s1 = spool.tile([BLK, 5 * IN_F], f32, tag="s1", name=f"s1_{b}")
            s2 = spool.tile([BLK, 2 * IN_F], f32, tag="s2", name=f"s2_{b}")
            hs = spool.tile([BLK, IN_F], f32, tag="hs", name=f"hs_{b}")
            nc.vector.tensor_add(s1[:], v[:, :, 0, :], v[:, :, 1, :])
            nc.vector.tensor_add(s2[:], s1[:, 0:2 * IN_F], s1[:, 2 * IN_F:4 * IN_F])
            nc.vector.tensor_add(s2[:, 0:IN_F], s2[:, 0:IN_F], s2[:, IN_F:2 * IN_F])
            nc.vector.tensor_add(hs[:], s2[:, 0:IN_F], s1[:, 4 * IN_F:5 * IN_F])
            return emit_transposes(b, hs)

        def emit_w1_relu_pair(b, hhT):
            """W1 matmul + ReLU + hop-2 pair-add for block b."""
            h1b = [h1pool.tile([128, BLK], bf16, tag=f"h1b{hc}",
                               name=f"h1b{hc}_{b}") for hc in range(2)]
            for hc in range(2):
                pm = ps_mm.tile([128, 512], f32, tag="pm", space="PSUM")
                for fc in range(4):
                    nc.tensor.matmul(
                        pm[:, 0:BLK],
                        lhsT=w1t[:, fc * HID + hc * 128: fc * HID + hc * 128 + 128],
                        rhs=hhT[fc][:],
                        start=(fc == 0), stop=(fc == 3),
                    )
                nc.scalar.activation(h1b[hc][:], pm[:, 0:BLK],
                                     mybir.ActivationFunctionType.Relu,
                                     bias=b1t[:, hc:hc + 1])
            with nc.allow_low_precision(reason="bf16 pooling within 2e-2 tol"):
                for hc in range(2):
                    v = h1b[hc][:].rearrange("p (i two) -> p i two", two=2)
                    nc.vector.tensor_add(
                        pairT[hc][:, b * (BLK // 2):(b + 1) * (BLK // 2)],
                        v[:, :, 0], v[:, :, 1])

        def emit_chunk_tail(c):
            """5->1 pair-tree + W2 + store for dst chunk c (64 rows)."""
            p0 = c * CHUNK_BLKS * (BLK // 2)
            g2c = []
            with nc.allow_low_precision(reason="bf16 pooling within 2e-2 tol"):
                for hc in range(2):
                    v = pairT[hc][:, p0:p0 + 5 * CHUNK_DST].rearrange(
                        "p (r k) -> p r k", k=5)
                    ta = t2pool.tile([128, CHUNK_DST], bf16, tag="t2a",
                                     name=f"t2a_{c}{hc}")
                    tb = t2pool.tile([128, CHUNK_DST], bf16, tag="t2b",
                                     name=f"t2b_{c}{hc}")
                    gc = t2pool.tile([128, CHUNK_DST], bf16, tag=f"g2c{hc}",
                                     name=f"g2c{hc}_{c}")
                    nc.vector.tensor_add(ta[:], v[:, :, 0], v[:, :, 1])
                    nc.vector.tensor_add(tb[:], v[:, :, 2], v[:, :, 3])
                    nc.vector.tensor_add(ta[:], ta[:], tb[:])
                    nc.vector.tensor_add(gc[:], ta[:], v[:, :, 4])
                    g2c.append(gc)
            po = ps_o.tile([CHUNK_DST, 512], f32, tag="po", space="PSUM")
            for hc in range(2):
                nc.tensor.matmul(
                    po[:, 0:NCLS],
                    lhsT=g2c[hc][:, 0:CHUNK_DST],
                    rhs=w2t[:, hc * NCLS:(hc + 1) * NCLS],
                    start=(hc == 0), stop=(hc == 1),
                )
            ot = opool.tile([CHUNK_DST, NCLS], f32, tag="ot", name=f"ot_{c}")
            nc.vector.tensor_copy(ot[:], po[:, 0:NCLS])
            nc.sync.dma_start(
                out.ap()[c * CHUNK_DST:(c + 1) * CHUNK_DST, :], ot[:])

        # software-pipelined: block b's gathers+pool+transposes are emitted
        # before W1 of block b-1, so PE never stalls on DVE's hhT copies.
        # The LAST block (39) is split: its slots 0..7 are gathered and
        # pre-summed before block 38, so the post-final-gather critical chain
        # is just transfer + 2 small adds + transposes + W1 + chunk tail.
        prev_hhT = None
        for b in range(NBLK - 2):
            hhT = emit_gather_and_pt(b)
            if prev_hhT is not None:
                emit_w1_relu_pair(b - 1, prev_hhT)
                if b % CHUNK_BLKS == 0:
                    emit_chunk_tail(b // CHUNK_BLKS - 1)
            prev_hhT = hhT
        # last two blocks: slots 0..7 gathered + pre-summed early so only 4
        # tiny preps and short stub adds remain after the final descriptor-gen
        gt = {}
        s2a = {}
        for b in (NBLK - 2, NBLK - 1):
            g = gpool.tile([BLK, F * IN_F], f32, tag=f"g{b}", name=f"g_{b}",
                           bufs=1)
            gt[b] = g
            emit_gathers(g, b, 0, 8)
            v = g[:].rearrange("p (r two f) -> p r two f", two=2, f=IN_F)
            s1 = prpool.tile([BLK, 4 * IN_F], f32, tag=f"s{b}a", name=f"s{b}a")
            s2 = prpool.tile([BLK, 2 * IN_F], f32, tag=f"s{b}b", name=f"s{b}b")
            nc.vector.tensor_add(s1[:], v[:, 0:4, 0, :], v[:, 0:4, 1, :])
            nc.vector.tensor_add(s2[:], s1[:, 0:2 * IN_F],
                                 s1[:, 2 * IN_F:4 * IN_F])
            nc.vector.tensor_add(s2[:, 0:IN_F], s2[:, 0:IN_F],
                                 s2[:, IN_F:2 * IN_F])
            s2a[b] = s2
        emit_w1_relu_pair(NBLK - 3, prev_hhT)
        hhTl = {}
        for b in (NBLK - 2, NBLK - 1):
            emit_gathers(gt[b], b, 8, F)
            g = gt[b]
            p89 = prpool.tile([BLK, IN_F], f32, tag=f"s{b}c", name=f"s{b}c")
            hs = prpool.tile([BLK, IN_F], f32, tag=f"s{b}d", name=f"s{b}d")
            nc.vector.tensor_add(p89[:], g[:, 8 * IN_F:9 * IN_F],
                                 g[:, 9 * IN_F:10 * IN_F])
            nc.vector.tensor_add(hs[:], s2a[b][:, 0:IN_F], p89[:])
            hhTl[b] = emit_transposes(b, hs)
        emit_w1_relu_pair(NBLK - 2, hhTl[NBLK - 2])
        emit_w1_relu_pair(NBLK - 1, hhTl[NBLK - 1])
        emit_chunk_tail(NCHUNK - 1)

    return nc


def _get_nc():
    global _BUILT
    if _BUILT is None:
        _BUILT = build_nc()
    return _BUILT


def _prep_core_indices(src0, src1, core):
    s1 = src1[core * GRP_PC:(core + 1) * GRP_PC].astype(np.int64)
    G = src0[(s1[:, None] * F + np.arange(F)[None, :])]        # [5120, 10]
    return np.ascontiguousarray(
        G.reshape(NBLK, BLK, F).transpose(1, 0, 2).reshape(BLK, NBLK * F)
    ).astype(np.int32)


def _prep_weights(W1, b1, W2):
    import ml_dtypes
    w1t = (np.asarray(W1, dtype=np.float32) / np.float32(F)).reshape(
        4, 128, HID).transpose(1, 0, 2).reshape(128, 4 * HID)
    w2t = (np.asarray(W2, dtype=np.float32) / np.float32(F)).reshape(
        2, 128, NCLS).transpose(1, 0, 2).reshape(128, 2 * NCLS)
    b1t = np.asarray(b1, dtype=np.float32).reshape(2, 128).T
    identb = np.eye(128, dtype=np.float32)
    return {
        "w1t": np.ascontiguousarray(w1t).astype(ml_dtypes.bfloat16),
        "w2t": np.ascontiguousarray(w2t).astype(ml_dtypes.bfloat16),
        "b1t": np.ascontiguousarray(b1t),
        "identb": identb,
    }


def _run(inputs, trace=False, trace_kwargs=None):
    from concourse.bass_utils import run_bass_kernel_spmd

    features = np.ascontiguousarray(inputs["features"], dtype=np.float32)
    b2 = np.ascontiguousarray(inputs["b2"], dtype=np.float32)
    src0 = np.asarray(inputs["src0"]).astype(np.int64)
    src1 = np.asarray(inputs["src1"]).astype(np.int64)
    wts = _prep_weights(inputs["W1"], inputs["b1"], inputs["W2"])

    in_maps = []
    for c in range(NC_N):
        idx = _prep_core_indices(src0, src1, c)
        in_maps.append({
            "feat": features,
            "idx0": np.ascontiguousarray(idx[:, :F]),
            "idxr": np.ascontiguousarray(idx[:, F:]),
            **wts,
        })
    nc = _get_nc()
    kw = {}
    if trace:
        kw = {"trace": True, "trace_kwargs": trace_kwargs or {}}
    res = run_bass_kernel_spmd(nc, in_maps, list(range(NC_N)), **kw)
    full = np.concatenate([res.results[c]["out"] for c in range(NC_N)], axis=0)
    full = full + b2[None, :]
    return full, res


def kernel(features, W1, b1, W2, b2, src0, dst0, src1, dst1):
    ins = dict(features=features, W1=W1, b1=b1, W2=W2, b2=b2,
               src0=src0, dst0=dst0, src1=src1, dst1=dst1)
    d0 = np.asarray(dst0); d1 = np.asarray(dst1)
    fixed = (d0 == np.arange(N1 * F) // F).all() and \
            (d1 == np.arange(N2 * F) // F).all()
    if not fixed:
        # general (unexpected) dst pattern: numpy fallback for correctness
        f = np.asarray(features, dtype=np.float64)
        m = f[np.asarray(src0)]
        s = np.zeros((N1, IN_F)); np.add.at(s, d0, m)
        deg = np.bincount(d0, minlength=N1).clip(1)
        h = np.maximum(s / deg[:, None] @ np.asarray(W1) + np.asarray(b1), 0)
        m = h[np.asarray(src1)]
        s = np.zeros((N2, HID)); np.add.at(s, d1, m)
        deg = np.bincount(d1, minlength=N2).clip(1)
        return ((s / deg[:, None]) @ np.asarray(W2) + np.asarray(b2)
                ).astype(np.float32)
    out, _ = _run(ins)
    return out


# revision 13
# speedup vs baseline: 1.0631x; 1.0046x over previous
"""GCN sampling (NodeFlow) kernel for 8 TRN2 NeuronCores — v3.

Geometry (hardcoded per problem spec):
  N0=409600 nodes x 512 feats, layer0: 40960 dst x fanout 10, W1 [512,256]+relu,
  layer1: 4096 dst x fanout 10, W2 [256,64].

Strategy: shard layer-1 dst nodes across 8 cores (512 each). Each core pulls,
for each of its 5120 layer-1 edges, the 10 layer-0 feature rows of that edge's
src h-row (indices precomputed on host; h-rows deliberately duplicated per
layer-1 edge so BOTH segment-means become fixed-stride pooling, no on-device
gather for layer 1 and no cross-core communication). 1/10 mean factors are
folded into W1, W2 on the host.

The modeled bottleneck is the Pool engine's SWDGE descriptor generation for
the 400 indirect gathers (the HW DGE honors exactly one index per partition
per instruction — verified by probe; multi-index gathers return garbage for
slots >0, and the InstDMAGatherAnt custom path crashes this runtime). Per
gather: ~1040ns descriptor-gen vs 728ns DMA transfer, so ~415us Pool-bound.
Everything else is arranged to hide under it and to keep the head/tail of the
pipeline short:
  * block-0 indices are loaded as a separate tiny DMA so the first gather
    starts as early as possible; all constants load via HWDGE (SP engine),
    keeping Pool exclusively on gather descriptor generation.
  * per block of 128 edges: DVE pairwise-tree pools the 10 slots (f32),
    4 single-shot PE transposes produce h0^T, DVE downcasts PSUM->bf16,
    W1 matmul in bf16 (1 cycle/row) + ReLU per block.
  * hop-2 pooling is a per-block stride-2 pair-add (pairs never straddle
    blocks) plus a 5-block chunk tail (64 dst): 5->1 tree, W2 (bf16), store.
    The post-last-gather critical chain is one block's compute + one small
    chunk tail instead of a full-width epilogue.
"""

import sys

sys.path.insert(0, "/opt/trn_rl_repo")

from contextlib import ExitStack

import numpy as np

N0, N1, N2 = 409600, 40960, 4096
F = 10                      # fanout
IN_F, HID, NCLS = 512, 256, 64
NC_N = 8                    # cores
DST_PC = N2 // NC_N         # 512 dst nodes per core
GRP_PC = DST_PC * F         # 5120 h-rows (edges) per core
BLK = 128                   # edges per block (partition dim)
NBLK = GRP_PC // BLK        # 40 blocks
CHUNK_BLKS = 5              # blocks per output chunk (640 edges = 64 dst)
NCHUNK = NBLK // CHUNK_BLKS  # 8 output chunks
CHUNK_DST = BLK * CHUNK_BLKS // F   # 64 dst rows per chunk

_BUILT = None


def _legalize_waits(bir: bytes) -> bytes:
    """This container's walrus supports exactly ONE sync-wait per instruction.
    Split every multi-wait instruction: keep the last wait, hoist the others
    onto single-wait EventSemaphore instructions inserted just before it on
    the same engine (same semantics: engine sequencer blocks in order)."""
    import orjson

    j = orjson.loads(bir)
    n_new = 0
    for fn in j["functions"]:
        for bb in fn["blocks"]:
            insts = bb["instructions"]
            out = []
            for inst in insts:
                si = inst.get("sync_info")
                waits = si.get("on_wait") if si else None
                if waits and len(waits) > 1:
                    for w in waits[:-1]:
                        n_new += 1
                        out.append({
                            "debug": inst.get("debug", 0),
                            "engine": inst["engine"],
                            "ins": [],
                            "name": f"{inst['name']}_esw{n_new}",
                            "opcode": "EventSemaphore",
                            "outs": [],
                            "sync_info": {"on_update": [], "on_wait": [w]},
                        })
                    si["on_wait"] = [waits[-1]]
                out.append(inst)
            bb["instructions"] = out
    return orjson.dumps(j)


def _install_patch():
    import concourse.bass as bass

    if getattr(bass.Bass, "_gcn_wait_patch", False):
        return
    orig = bass.Bass.to_json_bytes

    def to_json_bytes(self, *a, **kw):
        return _legalize_waits(orig(self, *a, **kw))

    bass.Bass.to_json_bytes = to_json_bytes
    bass.Bass._gcn_wait_patch = True


def build_nc():
    """Build the SPMD Bass program (identical on all cores)."""
    _install_patch()
    import concourse.bass as bass
    import concourse.tile as tile
    from concourse import mybir

    f32 = mybir.dt.float32
    bf16 = mybir.dt.bfloat16
    nc = bass.Bass("TRN2", target_bir_lowering=False, debug=False,
                   num_devices=NC_N, num_swdge_queues=4)

    feat = nc.dram_tensor("feat", [N0, IN_F], f32, kind="ExternalInput")
    # host-prearranged weights: w1t[p, fc*256+h] = W1[fc*128+p, h]/10 (bf16)
    w1t_d = nc.dram_tensor("w1t", [128, 4 * HID], bf16, kind="ExternalInput")
    # b1t[p, hc] = b1[hc*128+p] (f32)
    b1t_d = nc.dram_tensor("b1t", [128, 2], f32, kind="ExternalInput")
    # w2t[p, c*64+n] = W2[c*128+p, n]/10 (bf16)
    w2t_d = nc.dram_tensor("w2t", [128, 2 * NCLS], bf16, kind="ExternalInput")
    identb_d = nc.dram_tensor("identb", [128, 128], f32, kind="ExternalInput")
    idx0_d = nc.dram_tensor("idx0", [BLK, F], mybir.dt.int32,
                            kind="ExternalInput")
    idxr_d = nc.dram_tensor("idxr", [BLK, (NBLK - 1) * F], mybir.dt.int32,
                            kind="ExternalInput")
    out = nc.dram_tensor("out", [DST_PC, NCLS], f32, kind="ExternalOutput")

    with tile.TileContext(nc) as tc, ExitStack() as ctx:
        consts = ctx.enter_context(tc.tile_pool(name="consts", bufs=1))
        gpool = ctx.enter_context(tc.tile_pool(name="gather", bufs=3))
        spool = ctx.enter_context(tc.tile_pool(name="pooled", bufs=2))
        hpool = ctx.enter_context(tc.tile_pool(name="hhT", bufs=2))
        h1pool = ctx.enter_context(tc.tile_pool(name="h1b", bufs=2))
        prpool = ctx.enter_context(tc.tile_pool(name="pairs", bufs=1))
        t2pool = ctx.enter_context(tc.tile_pool(name="tail", bufs=2))
        opool = ctx.enter_context(tc.tile_pool(name="outs", bufs=2))
        ps_tr = ctx.enter_context(tc.tile_pool(name="ps_tr", bufs=3, space="PSUM"))
        ps_mm = ctx.enter_context(tc.tile_pool(name="ps_mm", bufs=2, space="PSUM"))
        ps_o = ctx.enter_context(tc.tile_pool(name="ps_o", bufs=2, space="PSUM"))

        # block-0 indices first (tiny), so gather 0 starts ~earliest
        idx0_t = consts.tile([BLK, F], mybir.dt.int32)
        nc.sync.dma_start(idx0_t[:], idx0_d.ap())
        idxr_t = consts.tile([BLK, (NBLK - 1) * F], mybir.dt.int32)
        nc.sync.dma_start(idxr_t[:], idxr_d.ap())
        w1t = consts.tile([128, 4 * HID], bf16)
        nc.sync.dma_start(w1t[:], w1t_d.ap())
        b1t = consts.tile([128, 2], f32)
        nc.sync.dma_start(b1t[:], b1t_d.ap())
        w2t = consts.tile([128, 2 * NCLS], bf16)
        nc.sync.dma_start(w2t[:], w2t_d.ap())
        identb = consts.tile([128, 128], f32)
        nc.sync.dma_start(identb[:], identb_d.ap())

        # pair-sums of hop-2 slots: pairT[hc][p, b*64+i] = h1[e=2i]+h1[e=2i+1]
        pairT = [prpool.tile([128, NBLK * BLK // 2], bf16, tag=f"pairT{hc}",
                             name=f"pairT{hc}") for hc in range(2)]

        def emit_gathers(g, b, k0, k1):
            """Single-index gathers for block b, slots [k0, k1)."""
            for k in range(k0, k1):
                if b == 0:
                    iap = idx0_t[:, k:k + 1]
                else:
                    iap = idxr_t[:, (b - 1) * F + k:(b - 1) * F + k + 1]
                nc.gpsimd.indirect_dma_start(
                    out=g[:, k * IN_F:(k + 1) * IN_F], out_offset=None,
                    in_=feat.ap(),
                    in_offset=bass.IndirectOffsetOnAxis(ap=iap, axis=0),
                )

        def emit_transposes(b, hs):
            """4 single-shot PE transposes -> hhT (bf16 downcast on copy)."""
            hhT = [hpool.tile([128, BLK], bf16, tag=f"hhT{fc}",
                              name=f"hhT{fc}_{b}") for fc in range(4)]
            for fc in range(4):
                ptr = ps_tr.tile([128, 128], f32, tag="ptr", space="PSUM")
                nc.tensor.transpose(ptr[:], hs[:, fc * 128:(fc + 1) * 128],
                                    identb[:])
                nc.vector.tensor_copy(hhT[fc][:], ptr[:])
            return hhT

        def emit_gather_and_pt(b):
            """10 single-index gathers for block b + pool + transpose."""
            g = gpool.tile([BLK, F * IN_F], f32, tag="g", name=f"g_{b}")
            emit_gathers(g, b, 0, F)
            # DVE pairwise tree over the 10 slots (f32, wide strided ops)
            v = g[:].rearrange("p (r two f) -> p r two f", two=2, f=IN_F)
            s1 = spool.tile([BLK, 5 * IN_F], f32, tag="s1", name=f"s1_{b}")
            s2 = spool.tile([BLK, 2 * IN_F], f32, tag="s2", name=f"s2_{b}")
            hs = spool.tile([BLK, IN_F], f32, tag="hs", name=f"hs_{b}")
            nc.vector.tensor_add(s1[:], v[:, :, 0, :], v[:, :, 1, :])
            nc.vector.tensor_add(s2[:], s1[:, 0:2 * IN_F], s1[:, 2 * IN_F:4 * IN_F])
            nc.vector.tensor_add(s2[:, 0:IN_F], s2[:, 0:IN_F], s2[:, IN_F:2 * IN_F])
            nc.vector.tensor_add(hs[:], s2[:, 0:IN_F], s1[:, 4 * IN_F:5 * IN_F])
            return emit_transposes(b, hs)

        def emit_w1_relu_pair(b, hhT):
            """W1 matmul + ReLU + hop-2 pair-add for block b."""
            h1b = [h1pool.tile([128, BLK], bf16, tag=f"h1b{hc}",
                               name=f"h1b{hc}_{b}") for hc in range(2)]
            for hc in range(2):
                pm = ps_mm.tile([128, 512], f32, tag="pm", space="PSUM")
                for fc in range(4):
                    nc.tensor.matmul(
                        pm[:, 0:BLK],
                        lhsT=w1t[:, fc * HID + hc * 128: fc * HID + hc * 128 + 128],
                        rhs=hhT[fc][:],
                        start=(fc == 0), stop=(fc == 3),
                    )
                nc.scalar.activation(h1b[hc][:], pm[:, 0:BLK],
                                     mybir.ActivationFunctionType.Relu,
                                     bias=b1t[:, hc:hc + 1])
            with nc.allow_low_precision(reason="bf16 pooling within 2e-2 tol"):
                for hc in range(2):
                    v = h1b[hc][:].rearrange("p (i two) -> p i two", two=2)
                    nc.vector.tensor_add(
                        pairT[hc][:, b * (BLK // 2):(b + 1) * (BLK // 2)],
                        v[:, :, 0], v[:, :, 1])

        def emit_chunk_tail(c):
            """5->1 pair-tree + W2 + store for dst chunk c (64 rows)."""
            p0 = c * CHUNK_BLKS * (BLK // 2)
            g2c = []
            with nc.allow_low_precision(reason="bf16 pooling within 2e-2 tol"):
                for hc in range(2):
                    v = pairT[hc][:, p0:p0 + 5 * CHUNK_DST].rearrange(
                        "p (r k) -> p r k", k=5)
                    ta = t2pool.tile([128, CHUNK_DST], bf16, tag="t2a",
                                     name=f"t2a_{c}{hc}")
                    tb = t2pool.tile([128, CHUNK_DST], bf16, tag="t2b",
                                     name=f"t2b_{c}{hc}")
                    gc = t2pool.tile([128, CHUNK_DST], bf16, tag=f"g2c{hc}",
                                     name=f"g2c{hc}_{c}")
                    nc.vector.tensor_add(ta[:], v[:, :, 0], v[:, :, 1])
                    nc.vector.tensor_add(tb[:], v[:, :, 2], v[:, :, 3])
                    nc.vector.tensor_add(ta[:], ta[:], tb[:])
                    nc.vector.tensor_add(gc[:], ta[:], v[:, :, 4])
                    g2c.append(gc)
            po = ps_o.tile([CHUNK_DST, 512], f32, tag="po", space="PSUM")
            for hc in range(2):
                nc.tensor.matmul(
                    po[:, 0:NCLS],
                    lhsT=g2c[hc][:, 0:CHUNK_DST],
                    rhs=w2t[:, hc * NCLS:(hc + 1) * NCLS],
                    start=(hc == 0), stop=(hc == 1),
                )
            ot = opool.tile([CHUNK_DST, NCLS], f32, tag="ot", name=f"ot_{c}")
            nc.vector.tensor_copy(ot[:], po[:, 0:NCLS])
            nc.sync.dma_start(
                out.ap()[c * CHUNK_DST:(c + 1) * CHUNK_DST, :], ot[:])

        # software-pipelined: block b's gathers+pool+transposes are emitted
        # before W1 of block b-1, so PE never stalls on DVE's hhT copies.
        # The LAST block (39) is split: its slots 0..7 are gathered and
        # pre-summed before block 38, so the post-final-gather critical chain
        # is just transfer + 2 small adds + transposes + W1 + chunk tail.
        prev_hhT = None
        for b in range(NBLK - 2):
            hhT = emit_gather_and_pt(b)
            if prev_hhT is not None:
                emit_w1_relu_pair(b - 1, prev_hhT)
                if b % CHUNK_BLKS == 0:
                    emit_chunk_tail(b // CHUNK_BLKS - 1)
            prev_hhT = hhT
        # last two blocks: slots 0..7 gathered + pre-summed early so only 4
        # tiny preps and short stub adds remain after the final descriptor-gen
        gt = {}
        s2a = {}
        for b in (NBLK - 2, NBLK - 1):
            g = gpool.tile([BLK, F * IN_F], f32, tag=f"g{b}", name=f"g_{b}",
                           bufs=1)
            gt[b] = g
            emit_gathers(g, b, 0, 8)
            v = g[:].rearrange("p (r two f) -> p r two f", two=2, f=IN_F)
            s1 = prpool.tile([BLK, 4 * IN_F], f32, tag=f"s{b}a", name=f"s{b}a")
            s2 = prpool.tile([BLK, 2 * IN_F], f32, tag=f"s{b}b", name=f"s{b}b")
            nc.vector.tensor_add(s1[:], v[:, 0:4, 0, :], v[:, 0:4, 1, :])
            nc.vector.tensor_add(s2[:], s1[:, 0:2 * IN_F],
                                 s1[:, 2 * IN_F:4 * IN_F])
            nc.vector.tensor_add(s2[:, 0:IN_F], s2[:, 0:IN_F],
                                 s2[:, IN_F:2 * IN_F])
            s2a[b] = s2
        emit_w1_relu_pair(NBLK - 3, prev_hhT)

        def finish_last_block(b):
            """Last 2 gathers + stub adds + transposes for a pre-summed block."""
            emit_gathers(gt[b], b, 8, F)
            g = gt[b]
            p89 = prpool.tile([BLK, IN_F], f32, tag=f"s{b}c", name=f"s{b}c")
            hs = prpool.tile([BLK, IN_F], f32, tag=f"s{b}d", name=f"s{b}d")
            nc.vector.tensor_add(p89[:], g[:, 8 * IN_F:9 * IN_F],
                                 g[:, 9 * IN_F:10 * IN_F])
            nc.vector.tensor_add(hs[:], s2a[b][:, 0:IN_F], p89[:])
            return emit_transposes(b, hs)

        hhT38 = finish_last_block(NBLK - 2)
        emit_w1_relu_pair(NBLK - 2, hhT38)
        hhT39 = finish_last_block(NBLK - 1)
        emit_w1_relu_pair(NBLK - 1, hhT39)
        emit_chunk_tail(NCHUNK - 1)

    return nc


def _get_nc():
    global _BUILT
    if _BUILT is None:
        _BUILT = build_nc()
    return _BUILT


def _prep_core_indices(src0, src1, core):
    s1 = src1[core * GRP_PC:(core + 1) * GRP_PC].astype(np.int64)
    G = src0[(s1[:, None] * F + np.arange(F)[None, :])]        # [5120, 10]
    return np.ascontiguousarray(
        G.reshape(NBLK, BLK, F).transpose(1, 0, 2).reshape(BLK, NBLK * F)
    ).astype(np.int32)


def _prep_weights(W1, b1, W2):
    import ml_dtypes
    w1t = (np.asarray(W1, dtype=np.float32) / np.float32(F)).reshape(
        4, 128, HID).transpose(1, 0, 2).reshape(128, 4 * HID)
    w2t = (np.asarray(W2, dtype=np.float32) / np.float32(F)).reshape(
        2, 128, NCLS).transpose(1, 0, 2).reshape(128, 2 * NCLS)
    b1t = np.asarray(b1, dtype=np.float32).reshape(2, 128).T
    identb = np.eye(128, dtype=np.float32)
    return {
        "w1t": np.ascontiguousarray(w1t).astype(ml_dtypes.bfloat16),
        "w2t": np.ascontiguousarray(w2t).astype(ml_dtypes.bfloat16),
        "b1t": np.ascontiguousarray(b1t),
        "identb": identb,
    }


def _run(inputs, trace=False, trace_kwargs=None):
    from concourse.bass_utils import run_bass_kernel_spmd

    features = np.ascontiguousarray(inputs["features"], dtype=np.float32)
    b2 = np.ascontiguousarray(inputs["b2"], dtype=np.float32)
    src0 = np.asarray(inputs["src0"]).astype(np.int64)
    src1 = np.asarray(inputs["src1"]).astype(np.int64)
    wts = _prep_weights(inputs["W1"], inputs["b1"], inputs["W2"])

    in_maps = []
    for c in range(NC_N):
        idx = _prep_core_indices(src0, src1, c)
        in_maps.append({
            "feat": features,
            "idx0": np.ascontiguousarray(idx[:, :F]),
            "idxr": np.ascontiguousarray(idx[:, F:]),
            **wts,
        })
    nc = _get_nc()
    kw = {}
    if trace:
        kw = {"trace": True, "trace_kwargs": trace_kwargs or {}}
    res = run_bass_kernel_spmd(nc, in_maps, list(range(NC_N)), **kw)
    full = np.concatenate([res.results[c]["out"] for c in range(NC_N)], axis=0)
    full = full + b2[None, :]
    return full, res


def kernel(features, W1, b1, W2, b2, src0, dst0, src1, dst1):
    ins = dict(features=features, W1=W1, b1=b1, W2=W2, b2=b2,
               src0=src0, dst0=dst0, src1=src1, dst1=dst1)
    d0 = np.asarray(dst0); d1 = np.asarray(dst1)
    fixed = (d0 == np.arange(N1 * F) // F).all() and \
            (d1 == np.arange(N2 * F) // F).all()
    if not fixed:
        # general (unexpected) dst pattern: numpy fallback for correctness
        f = np.asarray(features, dtype=np.float64)
        m = f[np.asarray(src0)]
        s = np.zeros((N1, IN_F)); np.add.at(s, d0, m)
        deg = np.bincount(d0, minlength=N1).clip(1)
        h = np.maximum(s / deg[:, None] @ np.asarray(W1) + np.asarray(b1), 0)
        m = h[np.asarray(src1)]
        s = np.zeros((N2, HID)); np.add.at(s, d1, m)
        deg = np.bincount(d1, minlength=N2).clip(1)
        return ((s / deg[:, None]) @ np.asarray(W2) + np.asarray(b2)
                ).astype(np.float32)
    out, _ = _run(ins)
    return out


# revision 15
# speedup vs baseline: 1.1026x; 1.0372x over previous
"""GCN sampling (NodeFlow) kernel for 8 TRN2 NeuronCores — v3.

Geometry (hardcoded per problem spec):
  N0=409600 nodes x 512 feats, layer0: 40960 dst x fanout 10, W1 [512,256]+relu,
  layer1: 4096 dst x fanout 10, W2 [256,64].

Strategy: shard layer-1 dst nodes across 8 cores (512 each). Each core pulls,
for each of its 5120 layer-1 edges, the 10 layer-0 feature rows of that edge's
src h-row (indices precomputed on host; h-rows deliberately duplicated per
layer-1 edge so BOTH segment-means become fixed-stride pooling, no on-device
gather for layer 1 and no cross-core communication). 1/10 mean factors are
folded into W1, W2 on the host.

The modeled bottleneck is the Pool engine's SWDGE descriptor generation for
the 400 indirect gathers (the HW DGE honors exactly one index per partition
per instruction — verified by probe; multi-index gathers return garbage for
slots >0, and the InstDMAGatherAnt custom path crashes this runtime). Per
gather: ~1040ns descriptor-gen vs 728ns DMA transfer, so ~415us Pool-bound.
Everything else is arranged to hide under it and to keep the head/tail of the
pipeline short:
  * block-0 indices are loaded as a separate tiny DMA so the first gather
    starts as early as possible; all constants load via HWDGE (SP engine),
    keeping Pool exclusively on gather descriptor generation.
  * per block of 128 edges: DVE pairwise-tree pools the 10 slots (f32),
    4 single-shot PE transposes produce h0^T, DVE downcasts PSUM->bf16,
    W1 matmul in bf16 (1 cycle/row) + ReLU per block.
  * hop-2 pooling is a per-block stride-2 pair-add (pairs never straddle
    blocks) plus a 5-block chunk tail (64 dst): 5->1 tree, W2 (bf16), store.
    The post-last-gather critical chain is one block's compute + one small
    chunk tail instead of a full-width epilogue.
"""

import sys

sys.path.insert(0, "/opt/trn_rl_repo")

from contextlib import ExitStack

import numpy as np

N0, N1, N2 = 409600, 40960, 4096
F = 10                      # fanout
IN_F, HID, NCLS = 512, 256, 64
NC_N = 8                    # cores
DST_PC = N2 // NC_N         # 512 dst nodes per core
GRP_PC = DST_PC * F         # 5120 h-rows (edges) per core
BLK = 128                   # edges per block (partition dim)
NBLK = GRP_PC // BLK        # 40 blocks
CHUNK_BLKS = 5              # blocks per output chunk (640 edges = 64 dst)
NCHUNK = NBLK // CHUNK_BLKS  # 8 output chunks
CHUNK_DST = BLK * CHUNK_BLKS // F   # 64 dst rows per chunk

_BUILT = None


NBLK_D = 38                 # slot blocks after dedup
UB = NBLK_D * BLK           # 4864 slots
NSC = 10                    # superchunks of <=512 slot-cols
SENT = 600                  # f-table sentinel: no referencing dst

def _legalize_waits(bir: bytes) -> bytes:
    """This container's walrus supports exactly ONE sync-wait per instruction.
    Split every multi-wait instruction: keep the last wait, hoist the others
    onto single-wait EventSemaphore instructions inserted just before it on
    the same engine (same semantics: engine sequencer blocks in order)."""
    import orjson

    j = orjson.loads(bir)
    n_new = 0
    for fn in j["functions"]:
        for bb in fn["blocks"]:
            insts = bb["instructions"]
            out = []
            for inst in insts:
                si = inst.get("sync_info")
                waits = si.get("on_wait") if si else None
                if waits and len(waits) > 1:
                    for w in waits[:-1]:
                        n_new += 1
                        out.append({
                            "debug": inst.get("debug", 0),
                            "engine": inst["engine"],
                            "ins": [],
                            "name": f"{inst['name']}_esw{n_new}",
                            "opcode": "EventSemaphore",
                            "outs": [],
                            "sync_info": {"on_update": [], "on_wait": [w]},
                        })
                    si["on_wait"] = [waits[-1]]
                out.append(inst)
            bb["instructions"] = out
    return orjson.dumps(j)


def _install_patch():
    import concourse.bass as bass

    if getattr(bass.Bass, "_gcn_wait_patch", False):
        return
    orig = bass.Bass.to_json_bytes

    def to_json_bytes(self, *a, **kw):
        return _legalize_waits(orig(self, *a, **kw))

    bass.Bass.to_json_bytes = to_json_bytes
    bass.Bass._gcn_wait_patch = True


def build_nc():
    _install_patch()
    import concourse.bass as bass
    import concourse.tile as tile
    from concourse import mybir

    f32 = mybir.dt.float32
    bf16 = mybir.dt.bfloat16
    i32 = mybir.dt.int32
    nc = bass.Bass("TRN2", target_bir_lowering=False, debug=False,
                   num_devices=NC_N, num_swdge_queues=4)

    feat = nc.dram_tensor("feat", [N0, IN_F], f32, kind="ExternalInput")
    w1t_d = nc.dram_tensor("w1t", [128, 4 * HID], bf16, kind="ExternalInput")
    b1t_d = nc.dram_tensor("b1t", [128, 2], f32, kind="ExternalInput")
    w2t_d = nc.dram_tensor("w2t", [128, 2 * NCLS], bf16, kind="ExternalInput")
    identb_d = nc.dram_tensor("identb", [128, 128], f32, kind="ExternalInput")
    identb2_d = nc.dram_tensor("identb2", [128, 128], bf16, kind="ExternalInput")
    idx0_d = nc.dram_tensor("idx0", [BLK, F], i32, kind="ExternalInput")
    idxr_d = nc.dram_tensor("idxr", [BLK, (NBLK_D - 1) * F], i32,
                            kind="ExternalInput")
    ft_d = nc.dram_tensor("ft", [BLK, 2 * NBLK_D], f32, kind="ExternalInput")
    iota_d = nc.dram_tensor("iota", [128, DST_PC], f32, kind="ExternalInput")
    out = nc.dram_tensor("out", [DST_PC, NCLS], f32, kind="ExternalOutput")

    with tile.TileContext(nc) as tc, ExitStack() as ctx:
        consts = ctx.enter_context(tc.tile_pool(name="consts", bufs=1))
        gpool = ctx.enter_context(tc.tile_pool(name="gather", bufs=3))
        spool = ctx.enter_context(tc.tile_pool(name="pooled", bufs=2))
        hpool = ctx.enter_context(tc.tile_pool(name="hhT", bufs=2))
        h1pool = ctx.enter_context(tc.tile_pool(name="h1T", bufs=1))
        wpool = ctx.enter_context(tc.tile_pool(name="w2s", bufs=2))
        apool = ctx.enter_context(tc.tile_pool(name="amat", bufs=2))
        opool = ctx.enter_context(tc.tile_pool(name="outs", bufs=2))
        ps_tr = ctx.enter_context(tc.tile_pool(name="ps_tr", bufs=2, space="PSUM"))
        ps_mm = ctx.enter_context(tc.tile_pool(name="ps_mm", bufs=2, space="PSUM"))
        ps_pw = ctx.enter_context(tc.tile_pool(name="ps_pw", bufs=1, space="PSUM"))
        ps_tw = ctx.enter_context(tc.tile_pool(name="ps_tw", bufs=1, space="PSUM"))
        ps_ot = ctx.enter_context(tc.tile_pool(name="ps_ot", bufs=1, space="PSUM"))
        ps_o = ctx.enter_context(tc.tile_pool(name="ps_o", bufs=1, space="PSUM"))

        idx0_t = consts.tile([BLK, F], i32)
        nc.sync.dma_start(idx0_t[:], idx0_d.ap())
        idxr_t = consts.tile([BLK, (NBLK_D - 1) * F], i32)
        nc.sync.dma_start(idxr_t[:], idxr_d.ap())
        w1t = consts.tile([128, 4 * HID], bf16)
        nc.sync.dma_start(w1t[:], w1t_d.ap())
        b1t = consts.tile([128, 2], f32)
        nc.sync.dma_start(b1t[:], b1t_d.ap())
        w2t = consts.tile([128, 2 * NCLS], bf16)
        nc.sync.dma_start(w2t[:], w2t_d.ap())
        identb = consts.tile([128, 128], f32)
        nc.sync.dma_start(identb[:], identb_d.ap())
        identb2 = consts.tile([128, 128], bf16)
        nc.sync.dma_start(identb2[:], identb2_d.ap())
        ft_t = consts.tile([BLK, 2 * NBLK_D], f32)
        nc.sync.dma_start(ft_t[:], ft_d.ap())
        iota_t = consts.tile([128, DST_PC], f32)
        nc.sync.dma_start(iota_t[:], iota_d.ap())

        # resident h1^T: [hid-half on partitions, slot-cols] bf16
        h1T = [h1pool.tile([128, UB], bf16, tag=f"h1T{hc}", name=f"h1T{hc}")
               for hc in range(2)]
        # persistent out^T accumulator [ncls, dst]
        outT = ps_ot.tile([NCLS, DST_PC], f32, tag="outT", space="PSUM")

        def emit_gathers(g, b, k0, k1):
            for k in range(k0, k1):
                if b == 0:
                    iap = idx0_t[:, k:k + 1]
                else:
                    iap = idxr_t[:, (b - 1) * F + k:(b - 1) * F + k + 1]
                nc.gpsimd.indirect_dma_start(
                    out=g[:, k * IN_F:(k + 1) * IN_F], out_offset=None,
                    in_=feat.ap(),
                    in_offset=bass.IndirectOffsetOnAxis(ap=iap, axis=0),
                )

        def emit_block_compute(b, hs):
            """transposes + W1 + relu into h1T for pooled block b."""
            hhT = [hpool.tile([128, BLK], bf16, tag=f"hhT{fc}",
                              name=f"hhT{fc}_{b}") for fc in range(4)]
            for fc in range(4):
                ptr = ps_tr.tile([128, 128], f32, tag="ptr", space="PSUM")
                nc.tensor.transpose(ptr[:], hs[:, fc * 128:(fc + 1) * 128],
                                    identb[:])
                nc.scalar.activation(hhT[fc][:], ptr[:],
                                     mybir.ActivationFunctionType.Copy)
            for hc in range(2):
                pm = ps_mm.tile([128, 512], f32, tag="pm", space="PSUM")
                for fc in range(4):
                    nc.tensor.matmul(
                        pm[:, 0:BLK],
                        lhsT=w1t[:, fc * HID + hc * 128: fc * HID + hc * 128 + 128],
                        rhs=hhT[fc][:],
                        start=(fc == 0), stop=(fc == 3),
                    )
                nc.scalar.activation(h1T[hc][:, b * BLK:(b + 1) * BLK],
                                     pm[:, 0:BLK],
                                     mybir.ActivationFunctionType.Relu,
                                     bias=b1t[:, hc:hc + 1])

        def emit_block(b):
            g = gpool.tile([BLK, F * IN_F], f32, tag="g", name=f"g_{b}")
            emit_gathers(g, b, 0, F)
            v = g[:].rearrange("p (r two f) -> p r two f", two=2, f=IN_F)
            s1 = spool.tile([BLK, 5 * IN_F], f32, tag="s1", name=f"s1_{b}")
            s2 = spool.tile([BLK, 2 * IN_F], f32, tag="s2", name=f"s2_{b}")
            hs = spool.tile([BLK, IN_F], f32, tag="hs", name=f"hs_{b}")
            nc.vector.tensor_add(s1[:], v[:, :, 0, :], v[:, :, 1, :])
            nc.vector.tensor_add(s2[:], s1[:, 0:2 * IN_F],
                                 s1[:, 2 * IN_F:4 * IN_F])
            nc.vector.tensor_add(s2[:, 0:IN_F], s2[:, 0:IN_F],
                                 s2[:, IN_F:2 * IN_F])
            nc.vector.tensor_add(hs[:], s2[:, 0:IN_F], s1[:, 4 * IN_F:5 * IN_F])
            emit_block_compute(b, hs)

        def emit_hop2_cols(c0, W):
            """h1w2 rows for slot-cols [c0, c0+W) and A/SEL matmuls."""
            s = c0 // 512
            pw = ps_pw.tile([NCLS, 512], f32, tag="pw", space="PSUM")
            for hc in range(2):
                nc.tensor.matmul(
                    pw[:, 0:W],
                    lhsT=w2t[:, hc * NCLS:(hc + 1) * NCLS],
                    rhs=h1T[hc][:, c0:c0 + W],
                    start=(hc == 0), stop=(hc == 1),
                )
            pwb = wpool.tile([NCLS, 512], bf16, tag="pwb", name=f"pwb_{s}")
            nc.scalar.activation(pwb[:, 0:W], pw[:, 0:W],
                                 mybir.ActivationFunctionType.Copy)
            for j in range(W // BLK):
                b = c0 // BLK + j
                ptw = ps_tw.tile([128, NCLS], bf16, tag="ptw", space="PSUM")
                nc.tensor.matmul(ptw[:], lhsT=pwb[:, j * BLK:(j + 1) * BLK],
                                 rhs=identb2[0:NCLS, 0:NCLS],
                                 start=True, stop=True, is_transpose=True)
                rws = wpool.tile([128, NCLS], bf16, tag="rws", name=f"rws_{b}")
                nc.scalar.activation(rws[:], ptw[:],
                                     mybir.ActivationFunctionType.Copy)
                for layer in range(2):
                    A = apool.tile([128, DST_PC], bf16, tag=f"A{layer}",
                                   name=f"A{layer}_{b}")
                    nc.vector.tensor_scalar(
                        A[:], iota_t[:], ft_t[:, 2 * b + layer:2 * b + layer + 1],
                        None, mybir.AluOpType.is_equal)
                    nc.tensor.matmul(
                        outT[:], lhsT=rws[:], rhs=A[:],
                        start=(b == 0 and layer == 0),
                        stop=(b == NBLK_D - 1 and layer == 1),
                    )

        for b in range(NBLK_D - 2):
            emit_block(b)
            if b % 4 == 3 and 7 <= b <= 35:
                emit_hop2_cols(512 * (b // 4 - 1), 512)

        # last two blocks with early slot-0..7 gathers + stub adds
        gt = {}
        s2a = {}
        for b in (NBLK_D - 2, NBLK_D - 1):
            g = gpool.tile([BLK, F * IN_F], f32, tag=f"g{b}", name=f"g_{b}",
                           bufs=1)
            gt[b] = g
            emit_gathers(g, b, 0, 8)
            v = g[:].rearrange("p (r two f) -> p r two f", two=2, f=IN_F)
            s1 = consts.tile([BLK, 4 * IN_F], f32, tag=f"s{b}a", name=f"s{b}a")
            s2 = consts.tile([BLK, 2 * IN_F], f32, tag=f"s{b}b", name=f"s{b}b")
            nc.vector.tensor_add(s1[:], v[:, 0:4, 0, :], v[:, 0:4, 1, :])
            nc.vector.tensor_add(s2[:], s1[:, 0:2 * IN_F],
                                 s1[:, 2 * IN_F:4 * IN_F])
            nc.vector.tensor_add(s2[:, 0:IN_F], s2[:, 0:IN_F],
                                 s2[:, IN_F:2 * IN_F])
            s2a[b] = s2
        emit_hop2_cols(512 * 8, 512)   # blocks 32-35, ready at loop end
        for b in (NBLK_D - 2, NBLK_D - 1):
            emit_gathers(gt[b], b, 8, F)
            g = gt[b]
            p89 = consts.tile([BLK, IN_F], f32, tag=f"s{b}c", name=f"s{b}c")
            hs = consts.tile([BLK, IN_F], f32, tag=f"s{b}d", name=f"s{b}d")
            nc.vector.tensor_add(p89[:], g[:, 8 * IN_F:9 * IN_F],
                                 g[:, 9 * IN_F:10 * IN_F])
            nc.vector.tensor_add(hs[:], s2a[b][:, 0:IN_F], p89[:])
            emit_block_compute(b, hs)
            emit_hop2_cols(b * BLK, BLK)

        # final: transpose outT -> [dst, ncls] and store
        ob = opool.tile([NCLS, DST_PC], bf16, tag="ob", name="ob")
        nc.scalar.activation(ob[:], outT[:],
                             mybir.ActivationFunctionType.Copy)
        for q in range(DST_PC // BLK):
            pf = ps_o.tile([128, NCLS], bf16, tag="pf", space="PSUM")
            nc.tensor.matmul(pf[:], lhsT=ob[:, q * BLK:(q + 1) * BLK],
                             rhs=identb2[0:NCLS, 0:NCLS],
                             start=True, stop=True, is_transpose=True)
            ot = opool.tile([128, NCLS], f32, tag="ot", name=f"ot_{q}")
            nc.vector.tensor_copy(ot[:], pf[:])
            nc.sync.dma_start(out.ap()[q * BLK:(q + 1) * BLK, :], ot[:])

    return nc


def _get_nc():
    global _BUILT
    if _BUILT is None:
        _BUILT = build_nc()
    return _BUILT


def _prep_core_dedup(src0, src1, core):
    """Slot list (<=2 referencing dsts per slot), gather idx + f-tables."""
    s = src1[core * GRP_PC:(core + 1) * GRP_PC].astype(np.int64)
    slots, f1, f2 = [], [], []
    open_slot = {}
    for e in range(GRP_PC):
        g = int(s[e]); d = e // F
        j = open_slot.get(g)
        if j is not None:
            f2[j] = d
            del open_slot[g]        # 3rd ref opens a new slot
        else:
            open_slot[g] = len(slots)
            slots.append(g); f1.append(d); f2.append(SENT)
    ns = len(slots)
    assert ns <= UB, f"core {core}: {ns} slots > {UB}"
    slots += [0] * (UB - ns)
    f1 += [SENT] * (UB - ns)
    f2 += [SENT] * (UB - ns)
    sl = np.asarray(slots, dtype=np.int64)
    G = src0[(sl[:, None] * F + np.arange(F)[None, :])]     # [UB, 10]
    idx = np.ascontiguousarray(
        G.reshape(NBLK_D, BLK, F).transpose(1, 0, 2).reshape(BLK, NBLK_D * F)
    ).astype(np.int32)
    ftab = np.stack([np.asarray(f1), np.asarray(f2)], axis=-1)  # [UB, 2]
    ft = np.ascontiguousarray(
        ftab.reshape(NBLK_D, BLK, 2).transpose(1, 0, 2).reshape(BLK, 2 * NBLK_D)
    ).astype(np.float32)
    return idx, ft


def _prep_weights(W1, b1, W2):
    import ml_dtypes
    w1t = (np.asarray(W1, dtype=np.float32) / np.float32(F)).reshape(
        4, 128, HID).transpose(1, 0, 2).reshape(128, 4 * HID)
    w2t = (np.asarray(W2, dtype=np.float32) / np.float32(F)).reshape(
        2, 128, NCLS).transpose(1, 0, 2).reshape(128, 2 * NCLS)
    b1t = np.asarray(b1, dtype=np.float32).reshape(2, 128).T
    identb = np.eye(128, dtype=np.float32)
    return {
        "w1t": np.ascontiguousarray(w1t).astype(ml_dtypes.bfloat16),
        "w2t": np.ascontiguousarray(w2t).astype(ml_dtypes.bfloat16),
        "b1t": np.ascontiguousarray(b1t),
        "identb": identb,
    }


def _run(inputs, trace=False, trace_kwargs=None):
    from concourse.bass_utils import run_bass_kernel_spmd

    features = np.ascontiguousarray(inputs["features"], dtype=np.float32)
    b2 = np.ascontiguousarray(inputs["b2"], dtype=np.float32)
    src0 = np.asarray(inputs["src0"]).astype(np.int64)
    src1 = np.asarray(inputs["src1"]).astype(np.int64)
    wts = _prep_weights(inputs["W1"], inputs["b1"], inputs["W2"])
    import ml_dtypes
    wts["identb2"] = np.eye(128, dtype=np.float32).astype(ml_dtypes.bfloat16)
    wts["iota"] = np.broadcast_to(np.arange(DST_PC, dtype=np.float32),
                                  (128, DST_PC)).copy()

    in_maps = []
    for c in range(NC_N):
        idx, ft = _prep_core_dedup(src0, src1, c)
        in_maps.append({
            "feat": features,
            "idx0": np.ascontiguousarray(idx[:, :F]),
            "idxr": np.ascontiguousarray(idx[:, F:]),
            "ft": ft,
            **wts,
        })
    nc = _get_nc()
    kw = {}
    if trace:
        kw = {"trace": True, "trace_kwargs": trace_kwargs or {}}
    res = run_bass_kernel_spmd(nc, in_maps, list(range(NC_N)), **kw)
    full = np.concatenate([res.results[c]["out"] for c in range(NC_N)], axis=0)
    full = full + b2[None, :]
    return full, res


def kernel(features, W1, b1, W2, b2, src0, dst0, src1, dst1):
    ins = dict(features=features, W1=W1, b1=b1, W2=W2, b2=b2,
               src0=src0, dst0=dst0, src1=src1, dst1=dst1)
    d0 = np.asarray(dst0); d1 = np.asarray(dst1)
    fixed = (d0 == np.arange(N1 * F) // F).all() and \
            (d1 == np.arange(N2 * F) // F).all()
    if not fixed:
        # general (unexpected) dst pattern: numpy fallback for correctness
        f = np.asarray(features, dtype=np.float64)
        m = f[np.asarray(src0)]
        s = np.zeros((N1, IN_F)); np.add.at(s, d0, m)
        deg = np.bincount(d0, minlength=N1).clip(1)
        h = np.maximum(s / deg[:, None] @ np.asarray(W1) + np.asarray(b1), 0)
        m = h[np.asarray(src1)]
        s = np.zeros((N2, HID)); np.add.at(s, d1, m)
        deg = np.bincount(d1, minlength=N2).clip(1)
        return ((s / deg[:, None]) @ np.asarray(W2) + np.asarray(b2)
                ).astype(np.float32)
    out, _ = _run(ins)
    return out


# revision 16
# speedup vs baseline: 1.1087x; 1.0055x over previous
"""GCN sampling (NodeFlow) kernel for 8 TRN2 NeuronCores — v3.

Geometry (hardcoded per problem spec):
  N0=409600 nodes x 512 feats, layer0: 40960 dst x fanout 10, W1 [512,256]+relu,
  layer1: 4096 dst x fanout 10, W2 [256,64].

Strategy: shard layer-1 dst nodes across 8 cores (512 each). Each core pulls,
for each of its 5120 layer-1 edges, the 10 layer-0 feature rows of that edge's
src h-row (indices precomputed on host; h-rows deliberately duplicated per
layer-1 edge so BOTH segment-means become fixed-stride pooling, no on-device
gather for layer 1 and no cross-core communication). 1/10 mean factors are
folded into W1, W2 on the host.

The modeled bottleneck is the Pool engine's SWDGE descriptor generation for
the 400 indirect gathers (the HW DGE honors exactly one index per partition
per instruction — verified by probe; multi-index gathers return garbage for
slots >0, and the InstDMAGatherAnt custom path crashes this runtime). Per
gather: ~1040ns descriptor-gen vs 728ns DMA transfer, so ~415us Pool-bound.
Everything else is arranged to hide under it and to keep the head/tail of the
pipeline short:
  * block-0 indices are loaded as a separate tiny DMA so the first gather
    starts as early as possible; all constants load via HWDGE (SP engine),
    keeping Pool exclusively on gather descriptor generation.
  * per block of 128 edges: DVE pairwise-tree pools the 10 slots (f32),
    4 single-shot PE transposes produce h0^T, DVE downcasts PSUM->bf16,
    W1 matmul in bf16 (1 cycle/row) + ReLU per block.
  * hop-2 pooling is a per-block stride-2 pair-add (pairs never straddle
    blocks) plus a 5-block chunk tail (64 dst): 5->1 tree, W2 (bf16), store.
    The post-last-gather critical chain is one block's compute + one small
    chunk tail instead of a full-width epilogue.
"""

import sys

sys.path.insert(0, "/opt/trn_rl_repo")

from contextlib import ExitStack

import numpy as np

N0, N1, N2 = 409600, 40960, 4096
F = 10                      # fanout
IN_F, HID, NCLS = 512, 256, 64
NC_N = 8                    # cores
DST_PC = N2 // NC_N         # 512 dst nodes per core
GRP_PC = DST_PC * F         # 5120 h-rows (edges) per core
BLK = 128                   # edges per block (partition dim)
NBLK = GRP_PC // BLK        # 40 blocks
CHUNK_BLKS = 5              # blocks per output chunk (640 edges = 64 dst)
NCHUNK = NBLK // CHUNK_BLKS  # 8 output chunks
CHUNK_DST = BLK * CHUNK_BLKS // F   # 64 dst rows per chunk

_BUILT = None


NBLK_D = 38                 # slot blocks after dedup
UB = NBLK_D * BLK           # 4864 slots
NSC = 10                    # superchunks of <=512 slot-cols
SENT = 600                  # f-table sentinel: no referencing dst

def _legalize_waits(bir: bytes) -> bytes:
    """This container's walrus supports exactly ONE sync-wait per instruction.
    Split every multi-wait instruction: keep the last wait, hoist the others
    onto single-wait EventSemaphore instructions inserted just before it on
    the same engine (same semantics: engine sequencer blocks in order)."""
    import orjson

    j = orjson.loads(bir)
    n_new = 0
    for fn in j["functions"]:
        for bb in fn["blocks"]:
            insts = bb["instructions"]
            out = []
            for inst in insts:
                si = inst.get("sync_info")
                waits = si.get("on_wait") if si else None
                if waits and len(waits) > 1:
                    for w in waits[:-1]:
                        n_new += 1
                        out.append({
                            "debug": inst.get("debug", 0),
                            "engine": inst["engine"],
                            "ins": [],
                            "name": f"{inst['name']}_esw{n_new}",
                            "opcode": "EventSemaphore",
                            "outs": [],
                            "sync_info": {"on_update": [], "on_wait": [w]},
                        })
                    si["on_wait"] = [waits[-1]]
                out.append(inst)
            bb["instructions"] = out
    return orjson.dumps(j)


def _install_patch():
    import concourse.bass as bass

    if getattr(bass.Bass, "_gcn_wait_patch", False):
        return
    orig = bass.Bass.to_json_bytes

    def to_json_bytes(self, *a, **kw):
        return _legalize_waits(orig(self, *a, **kw))

    bass.Bass.to_json_bytes = to_json_bytes
    bass.Bass._gcn_wait_patch = True


def build_nc(conservative=False):
    _install_patch()
    SPLIT_B, SPLIT_D = 30, 384
    import concourse.bass as bass
    import concourse.tile as tile
    from concourse import mybir

    f32 = mybir.dt.float32
    bf16 = mybir.dt.bfloat16
    i32 = mybir.dt.int32
    nc = bass.Bass("TRN2", target_bir_lowering=False, debug=False,
                   num_devices=NC_N, num_swdge_queues=4)

    feat = nc.dram_tensor("feat", [N0, IN_F], f32, kind="ExternalInput")
    w1t_d = nc.dram_tensor("w1t", [128, 4 * HID], bf16, kind="ExternalInput")
    b1t_d = nc.dram_tensor("b1t", [128, 2], f32, kind="ExternalInput")
    w2t_d = nc.dram_tensor("w2t", [128, 2 * NCLS], bf16, kind="ExternalInput")
    identb_d = nc.dram_tensor("identb", [128, 128], f32, kind="ExternalInput")
    identb2_d = nc.dram_tensor("identb2", [128, 128], bf16, kind="ExternalInput")
    idx0_d = nc.dram_tensor("idx0", [BLK, F], i32, kind="ExternalInput")
    idxr_d = nc.dram_tensor("idxr", [BLK, (NBLK_D - 1) * F], i32,
                            kind="ExternalInput")
    ft_d = nc.dram_tensor("ft", [BLK, 2 * NBLK_D], f32, kind="ExternalInput")
    iota_d = nc.dram_tensor("iota", [128, DST_PC], f32, kind="ExternalInput")
    out = nc.dram_tensor("out", [DST_PC, NCLS], f32, kind="ExternalOutput")

    with tile.TileContext(nc) as tc, ExitStack() as ctx:
        consts = ctx.enter_context(tc.tile_pool(name="consts", bufs=1))
        gpool = ctx.enter_context(tc.tile_pool(name="gather", bufs=3))
        spool = ctx.enter_context(tc.tile_pool(name="pooled", bufs=2))
        hpool = ctx.enter_context(tc.tile_pool(name="hhT", bufs=2))
        h1pool = ctx.enter_context(tc.tile_pool(name="h1T", bufs=1))
        wpool = ctx.enter_context(tc.tile_pool(name="w2s", bufs=2))
        apool = ctx.enter_context(tc.tile_pool(name="amat", bufs=2))
        opool = ctx.enter_context(tc.tile_pool(name="outs", bufs=2))
        ps_tr = ctx.enter_context(tc.tile_pool(name="ps_tr", bufs=1, space="PSUM"))
        ps_mm = ctx.enter_context(tc.tile_pool(name="ps_mm", bufs=2, space="PSUM"))
        ps_pw = ctx.enter_context(tc.tile_pool(name="ps_pw", bufs=1, space="PSUM"))
        ps_tw = ctx.enter_context(tc.tile_pool(name="ps_tw", bufs=1, space="PSUM"))
        ps_ot = ctx.enter_context(tc.tile_pool(name="ps_ot", bufs=1, space="PSUM"))
        ps_o = ctx.enter_context(tc.tile_pool(name="ps_o", bufs=1, space="PSUM"))

        idx0_t = consts.tile([BLK, F], i32)
        nc.sync.dma_start(idx0_t[:], idx0_d.ap())
        idxr_t = consts.tile([BLK, (NBLK_D - 1) * F], i32)
        nc.sync.dma_start(idxr_t[:], idxr_d.ap())
        w1t = consts.tile([128, 4 * HID], bf16)
        nc.sync.dma_start(w1t[:], w1t_d.ap())
        b1t = consts.tile([128, 2], f32)
        nc.sync.dma_start(b1t[:], b1t_d.ap())
        w2t = consts.tile([128, 2 * NCLS], bf16)
        nc.sync.dma_start(w2t[:], w2t_d.ap())
        identb = consts.tile([128, 128], f32)
        nc.sync.dma_start(identb[:], identb_d.ap())
        identb2 = consts.tile([128, 128], bf16)
        nc.sync.dma_start(identb2[:], identb2_d.ap())
        ft_t = consts.tile([BLK, 2 * NBLK_D], f32)
        nc.sync.dma_start(ft_t[:], ft_d.ap())
        iota_t = consts.tile([128, DST_PC], f32)
        nc.sync.dma_start(iota_t[:], iota_d.ap())

        preA = {}
        for pb in (NBLK_D - 2, NBLK_D - 1):
            for layer in range(2):
                pA = consts.tile([128, DST_PC], bf16, tag=f"pA{pb}_{layer}",
                                 name=f"pA{pb}_{layer}")
                nc.vector.tensor_scalar(
                    pA[:], iota_t[:],
                    ft_t[:, 2 * pb + layer:2 * pb + layer + 1],
                    None, mybir.AluOpType.is_equal)
                preA[(pb, layer)] = pA

        # resident h1^T: [hid-half on partitions, slot-cols] bf16
        h1T = [h1pool.tile([128, UB], bf16, tag=f"h1T{hc}", name=f"h1T{hc}")
               for hc in range(2)]
        # persistent out^T accumulators, split by dst range so the low-dst
        # stores can issue before the tail (first-ref dsts are monotone over
        # slots; host verifies blocks >= SPLIT_B only reference dst >= SPLIT_D)
        outT_A = ps_ot.tile([NCLS, SPLIT_D], f32, tag="outT_A", space="PSUM")
        outT_B = ps_ot.tile([NCLS, DST_PC - SPLIT_D], f32, tag="outT_B",
                            space="PSUM")
        A_LAST = NBLK_D - 1 if conservative else SPLIT_B - 1

        def emit_gathers(g, b, k0, k1):
            for k in range(k0, k1):
                if b == 0:
                    iap = idx0_t[:, k:k + 1]
                else:
                    iap = idxr_t[:, (b - 1) * F + k:(b - 1) * F + k + 1]
                nc.gpsimd.indirect_dma_start(
                    out=g[:, k * IN_F:(k + 1) * IN_F], out_offset=None,
                    in_=feat.ap(),
                    in_offset=bass.IndirectOffsetOnAxis(ap=iap, axis=0),
                )

        def emit_block_compute(b, hs):
            """transposes + W1 + relu into h1T for pooled block b."""
            hhT = [hpool.tile([128, BLK], bf16, tag=f"hhT{fc}",
                              name=f"hhT{fc}_{b}") for fc in range(4)]
            for fc in range(4):
                ptr = ps_tr.tile([128, 128], f32, tag="ptr", space="PSUM")
                nc.tensor.transpose(ptr[:], hs[:, fc * 128:(fc + 1) * 128],
                                    identb[:])
                nc.scalar.activation(hhT[fc][:], ptr[:],
                                     mybir.ActivationFunctionType.Copy)
            for hc in range(2):
                pm = ps_mm.tile([128, 512], f32, tag="pm", space="PSUM")
                for fc in range(4):
                    nc.tensor.matmul(
                        pm[:, 0:BLK],
                        lhsT=w1t[:, fc * HID + hc * 128: fc * HID + hc * 128 + 128],
                        rhs=hhT[fc][:],
                        start=(fc == 0), stop=(fc == 3),
                    )
                nc.scalar.activation(h1T[hc][:, b * BLK:(b + 1) * BLK],
                                     pm[:, 0:BLK],
                                     mybir.ActivationFunctionType.Relu,
                                     bias=b1t[:, hc:hc + 1])

        def emit_block(b):
            g = gpool.tile([BLK, F * IN_F], f32, tag="g", name=f"g_{b}")
            emit_gathers(g, b, 0, F)
            v = g[:].rearrange("p (r two f) -> p r two f", two=2, f=IN_F)
            s1 = spool.tile([BLK, 5 * IN_F], f32, tag="s1", name=f"s1_{b}")
            s2 = spool.tile([BLK, 2 * IN_F], f32, tag="s2", name=f"s2_{b}")
            hs = spool.tile([BLK, IN_F], f32, tag="hs", name=f"hs_{b}")
            nc.vector.tensor_add(s1[:], v[:, :, 0, :], v[:, :, 1, :])
            nc.vector.tensor_add(s2[:], s1[:, 0:2 * IN_F],
                                 s1[:, 2 * IN_F:4 * IN_F])
            nc.vector.tensor_add(s2[:, 0:IN_F], s2[:, 0:IN_F],
                                 s2[:, IN_F:2 * IN_F])
            nc.vector.tensor_add(hs[:], s2[:, 0:IN_F], s1[:, 4 * IN_F:5 * IN_F])
            emit_block_compute(b, hs)

        def emit_hop2_cols(c0, W):
            """h1w2 rows for slot-cols [c0, c0+W) and A/SEL matmuls."""
            s = c0 // 512
            pw = ps_pw.tile([NCLS, 512], f32, tag="pw", space="PSUM")
            for hc in range(2):
                nc.tensor.matmul(
                    pw[:, 0:W],
                    lhsT=w2t[:, hc * NCLS:(hc + 1) * NCLS],
                    rhs=h1T[hc][:, c0:c0 + W],
                    start=(hc == 0), stop=(hc == 1),
                )
            pwb = wpool.tile([NCLS, 512], bf16, tag="pwb", name=f"pwb_{s}")
            nc.scalar.activation(pwb[:, 0:W], pw[:, 0:W],
                                 mybir.ActivationFunctionType.Copy)
            for j in range(W // BLK):
                b = c0 // BLK + j
                ptw = ps_tw.tile([128, NCLS], bf16, tag="ptw", space="PSUM")
                nc.tensor.matmul(ptw[:], lhsT=pwb[:, j * BLK:(j + 1) * BLK],
                                 rhs=identb2[0:NCLS, 0:NCLS],
                                 start=True, stop=True, is_transpose=True)
                rws = wpool.tile([128, NCLS], bf16, tag="rws", name=f"rws_{b}")
                nc.scalar.activation(rws[:], ptw[:],
                                     mybir.ActivationFunctionType.Copy)
                for layer in range(2):
                    if (b, layer) in preA:
                        A = preA[(b, layer)]
                    else:
                        A = apool.tile([128, DST_PC], bf16, tag=f"A{layer}",
                                       name=f"A{layer}_{b}")
                        nc.vector.tensor_scalar(
                            A[:], iota_t[:],
                            ft_t[:, 2 * b + layer:2 * b + layer + 1],
                            None, mybir.AluOpType.is_equal)
                    if conservative or b < SPLIT_B:
                        nc.tensor.matmul(
                            outT_A[:], lhsT=rws[:], rhs=A[:, 0:SPLIT_D],
                            start=(b == 0 and layer == 0),
                            stop=(b == A_LAST and layer == 1),
                        )
                    nc.tensor.matmul(
                        outT_B[:], lhsT=rws[:], rhs=A[:, SPLIT_D:DST_PC],
                        start=(b == 0 and layer == 0),
                        stop=(b == NBLK_D - 1 and layer == 1),
                    )

        for b in range(NBLK_D - 2):
            emit_block(b)
            if b % 4 == 3 and 7 <= b <= 35:
                emit_hop2_cols(512 * (b // 4 - 1), 512)

        def emit_out(src_psum, d0, nd):
            obx = opool.tile([NCLS, 512], bf16, tag="ob", name=f"ob_{d0}")
            nc.scalar.activation(obx[:, 0:nd], src_psum[:],
                                 mybir.ActivationFunctionType.Copy)
            for q in range(nd // BLK):
                pf = ps_o.tile([128, NCLS], bf16, tag="pf", space="PSUM")
                nc.tensor.matmul(pf[:], lhsT=obx[:, q * BLK:(q + 1) * BLK],
                                 rhs=identb2[0:NCLS, 0:NCLS],
                                 start=True, stop=True, is_transpose=True)
                ot = opool.tile([128, NCLS], f32, tag="ot",
                                name=f"ot_{d0}_{q}")
                nc.vector.tensor_copy(ot[:], pf[:])
                nc.sync.dma_start(
                    out.ap()[d0 + q * BLK:d0 + (q + 1) * BLK, :], ot[:])

        # last two blocks with early slot-0..7 gathers + stub adds
        gt = {}
        s2a = {}
        for b in (NBLK_D - 2, NBLK_D - 1):
            g = gpool.tile([BLK, F * IN_F], f32, tag=f"g{b}", name=f"g_{b}",
                           bufs=1)
            gt[b] = g
            emit_gathers(g, b, 0, 8)
            v = g[:].rearrange("p (r two f) -> p r two f", two=2, f=IN_F)
            s1 = consts.tile([BLK, 4 * IN_F], f32, tag=f"s{b}a", name=f"s{b}a")
            s2 = consts.tile([BLK, 2 * IN_F], f32, tag=f"s{b}b", name=f"s{b}b")
            nc.vector.tensor_add(s1[:], v[:, 0:4, 0, :], v[:, 0:4, 1, :])
            nc.vector.tensor_add(s2[:], s1[:, 0:2 * IN_F],
                                 s1[:, 2 * IN_F:4 * IN_F])
            nc.vector.tensor_add(s2[:, 0:IN_F], s2[:, 0:IN_F],
                                 s2[:, IN_F:2 * IN_F])
            s2a[b] = s2
        if not conservative:
            emit_out(outT_A, 0, SPLIT_D)   # dsts < SPLIT_D final at block 29
        emit_hop2_cols(512 * 8, 512)   # blocks 32-35, ready at loop end
        for b in (NBLK_D - 2, NBLK_D - 1):
            emit_gathers(gt[b], b, 8, F)
            g = gt[b]
            p89 = consts.tile([BLK, IN_F], f32, tag=f"s{b}c", name=f"s{b}c")
            hs = consts.tile([BLK, IN_F], f32, tag=f"s{b}d", name=f"s{b}d")
            nc.vector.tensor_add(p89[:], g[:, 8 * IN_F:9 * IN_F],
                                 g[:, 9 * IN_F:10 * IN_F])
            nc.vector.tensor_add(hs[:], s2a[b][:, 0:IN_F], p89[:])
            emit_block_compute(b, hs)
            emit_hop2_cols(b * BLK, BLK)

        # final stores
        if conservative:
            emit_out(outT_A, 0, SPLIT_D)
        emit_out(outT_B, SPLIT_D, DST_PC - SPLIT_D)

    return nc


def _get_nc(conservative=False):
    global _BUILT
    if _BUILT is None:
        _BUILT = {}
    if conservative not in _BUILT:
        _BUILT[conservative] = build_nc(conservative)
    return _BUILT[conservative]


def _prep_core_dedup(src0, src1, core):
    """Slot list (<=2 referencing dsts per slot), gather idx + f-tables."""
    s = src1[core * GRP_PC:(core + 1) * GRP_PC].astype(np.int64)
    slots, f1, f2 = [], [], []
    open_slot = {}
    for e in range(GRP_PC):
        g = int(s[e]); d = e // F
        j = open_slot.get(g)
        if j is not None:
            f2[j] = d
            del open_slot[g]        # 3rd ref opens a new slot
        else:
            open_slot[g] = len(slots)
            slots.append(g); f1.append(d); f2.append(SENT)
    ns = len(slots)
    assert ns <= UB, f"core {core}: {ns} slots > {UB}"
    slots += [0] * (UB - ns)
    f1 += [SENT] * (UB - ns)
    f2 += [SENT] * (UB - ns)
    sl = np.asarray(slots, dtype=np.int64)
    G = src0[(sl[:, None] * F + np.arange(F)[None, :])]     # [UB, 10]
    idx = np.ascontiguousarray(
        G.reshape(NBLK_D, BLK, F).transpose(1, 0, 2).reshape(BLK, NBLK_D * F)
    ).astype(np.int32)
    ftab = np.stack([np.asarray(f1), np.asarray(f2)], axis=-1)  # [UB, 2]
    ft = np.ascontiguousarray(
        ftab.reshape(NBLK_D, BLK, 2).transpose(1, 0, 2).reshape(BLK, 2 * NBLK_D)
    ).astype(np.float32)
    return idx, ft


def _prep_weights(W1, b1, W2):
    import ml_dtypes
    w1t = (np.asarray(W1, dtype=np.float32) / np.float32(F)).reshape(
        4, 128, HID).transpose(1, 0, 2).reshape(128, 4 * HID)
    w2t = (np.asarray(W2, dtype=np.float32) / np.float32(F)).reshape(
        2, 128, NCLS).transpose(1, 0, 2).reshape(128, 2 * NCLS)
    b1t = np.asarray(b1, dtype=np.float32).reshape(2, 128).T
    identb = np.eye(128, dtype=np.float32)
    return {
        "w1t": np.ascontiguousarray(w1t).astype(ml_dtypes.bfloat16),
        "w2t": np.ascontiguousarray(w2t).astype(ml_dtypes.bfloat16),
        "b1t": np.ascontiguousarray(b1t),
        "identb": identb,
    }


def _run(inputs, trace=False, trace_kwargs=None):
    from concourse.bass_utils import run_bass_kernel_spmd

    features = np.ascontiguousarray(inputs["features"], dtype=np.float32)
    b2 = np.ascontiguousarray(inputs["b2"], dtype=np.float32)
    src0 = np.asarray(inputs["src0"]).astype(np.int64)
    src1 = np.asarray(inputs["src1"]).astype(np.int64)
    wts = _prep_weights(inputs["W1"], inputs["b1"], inputs["W2"])
    import ml_dtypes
    wts["identb2"] = np.eye(128, dtype=np.float32).astype(ml_dtypes.bfloat16)
    wts["iota"] = np.broadcast_to(np.arange(DST_PC, dtype=np.float32),
                                  (128, DST_PC)).copy()

    in_maps = []
    conservative = False
    for c in range(NC_N):
        idx, ft = _prep_core_dedup(src0, src1, c)
        # blocks >= 30 must only reference dst >= 384 for the split-out build
        tail_f = ft[:, 2 * 30:]
        if (tail_f[tail_f < SENT] < 384).any():
            conservative = True
        in_maps.append({
            "feat": features,
            "idx0": np.ascontiguousarray(idx[:, :F]),
            "idxr": np.ascontiguousarray(idx[:, F:]),
            "ft": ft,
            **wts,
        })
    nc = _get_nc(conservative)
    kw = {}
    if trace:
        kw = {"trace": True, "trace_kwargs": trace_kwargs or {}}
    res = run_bass_kernel_spmd(nc, in_maps, list(range(NC_N)), **kw)
    full = np.concatenate([res.results[c]["out"] for c in range(NC_N)], axis=0)
    full = full + b2[None, :]
    return full, res


def kernel(features, W1, b1, W2, b2, src0, dst0, src1, dst1):
    ins = dict(features=features, W1=W1, b1=b1, W2=W2, b2=b2,
               src0=src0, dst0=dst0, src1=src1, dst1=dst1)
    d0 = np.asarray(dst0); d1 = np.asarray(dst1)
    fixed = (d0 == np.arange(N1 * F) // F).all() and \
            (d1 == np.arange(N2 * F) // F).all()
    if not fixed:
        # general (unexpected) dst pattern: numpy fallback for correctness
        f = np.asarray(features, dtype=np.float64)
        m = f[np.asarray(src0)]
        s = np.zeros((N1, IN_F)); np.add.at(s, d0, m)
        deg = np.bincount(d0, minlength=N1).clip(1)
        h = np.maximum(s / deg[:, None] @ np.asarray(W1) + np.asarray(b1), 0)
        m = h[np.asarray(src1)]
        s = np.zeros((N2, HID)); np.add.at(s, d1, m)
        deg = np.bincount(d1, minlength=N2).clip(1)
        return ((s / deg[:, None]) @ np.asarray(W2) + np.asarray(b2)
                ).astype(np.float32)
    out, _ = _run(ins)
    return out
